# revision 6
# baseline (speedup 1.0000x reference)
"""Complex DFT (512-pt) over rows of x = x_re + i*x_im, y = x @ W^T (complex).

Full inputs: x_re, x_im (8,16,256,512) f32; w_re, w_im (512,512) f32.
Full output: (8,16,256,512,2) f32  (re/im interleaved on last axis).

Strategy (fp16 on-device; tolerance gate is 2e-2, fp16 lands ~5e-4):
  Conjugate symmetry W[N-h] = conj(W[h]) -> only half the spectrum columns.
  The four half-spectrum real products collapse into TWO 512-wide matmuls
  against ONE shared rhs:
      RHS = [ C(h=0..256) | D(h=1..255) ]   (512 cols = exactly 1 PSUM bank)
      PA  = Xre @ RHS = [ P1(0..256) | Q1(1..255) ]
      PB  = Xim @ RHS = [ Q2(0..256) | P2(1..255) ]
  with C = Re(W), D = Im(W), P1=A@C, P2=B@D, Q1=A@D, Q2=B@C.
  The device stores PA|PB raw (fp16); the HOST does the +- spectrum
  combines in f32 (same byte count, same accuracy as combining on-device
  after an fp16 round, but frees DVE/ACT to just evacuate PSUM).
  Edge columns h=0 and h=256 are plain/alternating row-sums of x computed
  on host in f32.

  Shard batch dim (8) across 8 cores -> per core (4096,512)x(512,512) x2.
  PE mapping: psum[m=128, 512] += lhsT[k=128, m=128].T @ rhs[k=128, 512].
  ~36 junk warm-up matmuls run while input DMAs stream so the HAM clock
  gate is at 8/8 (2.4 GHz) when real matmuls start.  ACT evacuates PA,
  DVE evacuates PB, gpsimd stores 2-m-tile mega-tiles.
"""

import sys

sys.path.insert(0, "/opt/trn_rl_repo")

import numpy as np

import concourse.bass as bass
import concourse.mybir as mybir
import concourse.tile as tile
from concourse import bacc
from concourse.bass_utils import run_bass_kernel_spmd

N = 512          # DFT size
B = 8            # batch -> one per core
M = 4096         # rows per core (16*256)
K = N            # contraction per product
KT = K // 128    # 4 k-subtiles per product
MT = M // 128    # 32 m-tiles
MP = 2           # m-tiles per input DMA
MS = 2           # m-tiles per output DMA (mega-store)
OW = 1024        # output cols per m-tile (PA | PB)
NSPIN = 36       # warm-up matmuls

_F32 = mybir.dt.float32
_F16 = mybir.dt.float16


def _build_bass():
    nc = bacc.Bacc("TRN2", target_bir_lowering=False, debug=False, num_devices=B)
    # xt[i] holds m-tiles 2i, 2i+1: free dim = (half, kblock(8), 128 m)
    # kblocks 0..3 = x_re, 4..7 = x_im (each [128 k-in-block, 128 m] lhsT).
    xt_d = nc.dram_tensor("xt", [MT // MP, 128, MP * 8 * 128], _F16, kind="ExternalInput")
    w_d = nc.dram_tensor("w", [KT, 128, 512], _F16, kind="ExternalInput")
    out_d = nc.dram_tensor("out", [MT // MS, 128, MS * OW], _F16, kind="ExternalOutput")

    _copy = mybir.ActivationFunctionType.Copy

    with tile.TileContext(nc) as tc:
        with (
            tc.tile_pool(name="wpool", bufs=1) as wpool,
            tc.tile_pool(name="xpool", bufs=16) as xpool,
            tc.tile_pool(name="opool", bufs=4) as opool,
            tc.tile_pool(name="psum", bufs=3, space="PSUM") as pspool,
        ):
            ws = []
            for k in range(KT):
                wt = wpool.tile([128, 512], _F16, tag=f"w{k}", name=f"w{k}")
                nc.scalar.dma_start(wt[:], w_d[k][:])
                ws.append(wt)
            for mt2 in range(MT // MP):
                xs = xpool.tile([128, MP * 8 * 128], _F16, tag="xs")
                nc.sync.dma_start(xs[:], xt_d[mt2][:])
                for half in range(MP):
                    mt = mt2 * MP + half
                    if mt % MS == 0:
                        ot = opool.tile([128, MS * OW], _F16, tag="ot")
                    oo = (mt % MS) * OW
                    xb = half * 8 * 128
                    pa = pspool.tile([128, 512], _F32, tag="pa", name="pa")
                    pb = pspool.tile([128, 512], _F32, tag="pb", name="pb")
                    for k in range(KT):
                        nc.tensor.matmul(
                            pa[:],
                            xs[:, xb + k * 128 : xb + (k + 1) * 128],
                            ws[k][:],
                            start=(k == 0),
                            stop=(k == KT - 1),
                        )
                    for k in range(KT):
                        nc.tensor.matmul(
                            pb[:],
                            xs[:, xb + (4 + k) * 128 : xb + (5 + k) * 128],
                            ws[k][:],
                            start=(k == 0),
                            stop=(k == KT - 1),
                        )
                    # evacuate PSUM -> fp16 store tile; ACT takes PA, DVE PB
                    nc.scalar.activation(ot[:, oo : oo + 512], pa[:], _copy, bias=0.0)
                    nc.vector.tensor_scalar_add(ot[:, oo + 512 : oo + 1024], pb[:], 0.0)
                    if mt % MS == MS - 1:
                        nc.gpsimd.dma_start(out_d[mt // MS][:], ot[:])
    nc.compile()
    return nc


_cached = {}


def _get_bass(trace=False):
    if "nc" not in _cached:
        _cached["nc"] = _build_bass()
    return _cached["nc"]


def _prep_weights(w_re, w_im):
    w_re = np.asarray(w_re, np.float32)
    w_im = np.asarray(w_im, np.float32)
    rhs = np.empty((K, 512), np.float16)
    rhs[:, 0:257] = w_re[0:257].T
    rhs[:, 257:512] = w_im[1:256].T
    return np.ascontiguousarray(rhs.reshape(KT, 128, 512))


def _prep_x_core(xr, xi):
    # lhsT: xcat_t[k, m] with k = [re 512 | im 512]; tile to
    # (MT/2 pairs, 128 k-in-block, (half, kblock, 128 m))
    xcat_t = np.empty((2 * N, M), np.float16)
    xcat_t[:N] = xr.reshape(M, N).T
    xcat_t[N:] = xi.reshape(M, N).T
    xt = xcat_t.reshape(8, 128, MT // MP, MP, 128).transpose(2, 1, 3, 0, 4)
    return np.ascontiguousarray(xt).reshape(MT // MP, 128, MP * 8 * 128)


def kernel(x_re, x_im, w_re, w_im, _trace=False, _trace_kwargs=None):
    x_re = np.asarray(x_re, np.float32)
    x_im = np.asarray(x_im, np.float32)
    w = _prep_weights(w_re, w_im)
    in_maps = [
        {"xt": _prep_x_core(x_re[c], x_im[c]), "w": w} for c in range(B)
    ]
    nc = _get_bass(_trace)
    res = run_bass_kernel_spmd(
        nc, in_maps, list(range(B)), trace=_trace, **(_trace_kwargs or {})
    )
    # edge spectrum cols (h=0, 256) in full f32 on host: plain/alternating sums
    alt = np.empty(N, np.float32)
    alt[0::2] = 1.0
    alt[1::2] = -1.0
    e0_re = x_re.sum(-1)                  # (B,16,256)
    e0_im = x_im.sum(-1)
    e256_re = x_re @ alt
    e256_im = x_im @ alt
    out = np.empty((B, 16, 256, N, 2), np.float32)
    for c in range(B):
        slab = (
            res.results[c]["out"]
            .reshape(MT // MS, 128, MS, OW)
            .transpose(0, 2, 1, 3)
            .reshape(M, OW)
            .astype(np.float32)
        )
        # PA = [P1(0..256) | Q1(1..255)], PB = [Q2(0..256) | P2(1..255)]
        P1 = slab[:, 0:257]
        Q1 = slab[:, 257:512]
        Q2 = slab[:, 512:769]
        P2 = slab[:, 769:1024]
        y = np.empty((M, N, 2), np.float32)
        y[:, 1:256, 0] = P1[:, 1:256] - P2
        y[:, 1:256, 1] = Q1 + Q2[:, 1:256]
        y[:, 257:512, 0] = (P1[:, 1:256] + P2)[:, ::-1]
        y[:, 257:512, 1] = (Q2[:, 1:256] - Q1)[:, ::-1]
        y[:, 0, 0] = e0_re[c].ravel()
        y[:, 0, 1] = e0_im[c].ravel()
        y[:, 256, 0] = e256_re[c].ravel()
        y[:, 256, 1] = e256_im[c].ravel()
        out[c] = y.reshape(16, 256, N, 2)
    if _trace:
        kernel._last_result = res
    return out


# revision 8
# speedup vs baseline: 1.0257x; 1.0257x over previous
"""Complex DFT (512-pt) over rows of x = x_re + i*x_im, y = x @ W^T (complex).

Full inputs: x_re, x_im (8,16,256,512) f32; w_re, w_im (512,512) f32.
Full output: (8,16,256,512,2) f32  (re/im interleaved on last axis).

Strategy (fp16 on-device; tolerance gate is 2e-2, fp16 lands ~5e-4):
  Conjugate symmetry W[N-h] = conj(W[h]) -> only half the spectrum columns.
  The four half-spectrum real products collapse into TWO 512-wide matmuls
  against ONE shared rhs:
      RHS = [ C(h=0..256) | D(h=1..255) ]   (512 cols = exactly 1 PSUM bank)
      PA  = Xre @ RHS = [ P1(0..256) | Q1(1..255) ]
      PB  = Xim @ RHS = [ Q2(0..256) | P2(1..255) ]
  with C = Re(W), D = Im(W), P1=A@C, P2=B@D, Q1=A@D, Q2=B@C.
  The device stores PA|PB raw (fp16); the HOST does the +- spectrum
  combines in f32 (same byte count, same accuracy as combining on-device
  after an fp16 round, but frees DVE/ACT to just evacuate PSUM).
  Edge columns h=0 and h=256 are plain/alternating row-sums of x computed
  on host in f32.

  Shard batch dim (8) across 8 cores -> per core (4096,512)x(512,512) x2.
  PE mapping: psum[m=128, 512] += lhsT[k=128, m=128].T @ rhs[k=128, 512].
  ~36 junk warm-up matmuls run while input DMAs stream so the HAM clock
  gate is at 8/8 (2.4 GHz) when real matmuls start.  ACT evacuates PA,
  DVE evacuates PB, gpsimd stores 2-m-tile mega-tiles.
"""

import sys

sys.path.insert(0, "/opt/trn_rl_repo")

import numpy as np

import concourse.bass as bass
import concourse.mybir as mybir
import concourse.tile as tile
from concourse import bacc
from concourse.bass_utils import run_bass_kernel_spmd

N = 512          # DFT size
B = 8            # batch -> one per core
M = 4096         # rows per core (16*256)
K = N            # contraction per product
KT = K // 128    # 4 k-subtiles per product
MT = M // 128    # 32 m-tiles
MP = 4           # m-tiles per input DMA
MS = 2           # m-tiles per output DMA (mega-store)
OW = 1024        # output cols per m-tile (PA | PB)
NSPIN = 8        # warm-up matmuls (bridge DMA latency, keep HAM window busy)

_F32 = mybir.dt.float32
_F16 = mybir.dt.float16


def _build_bass():
    nc = bacc.Bacc("TRN2", target_bir_lowering=False, debug=False, num_devices=B)
    # xt[i] holds m-tiles 2i, 2i+1: free dim = (half, kblock(8), 128 m)
    # kblocks 0..3 = x_re, 4..7 = x_im (each [128 k-in-block, 128 m] lhsT).
    xt_d = nc.dram_tensor("xt", [MT // MP, 128, MP * 8 * 128], _F16, kind="ExternalInput")
    w_d = nc.dram_tensor("w", [KT, 128, 512], _F16, kind="ExternalInput")
    out_d = nc.dram_tensor("out", [MT // MS, 128, MS * OW], _F16, kind="ExternalOutput")

    _copy = mybir.ActivationFunctionType.Copy

    with tile.TileContext(nc) as tc:
        with (
            tc.tile_pool(name="wpool", bufs=1) as wpool,
            tc.tile_pool(name="xpool", bufs=8) as xpool,
            tc.tile_pool(name="opool", bufs=4) as opool,
            tc.tile_pool(name="psum", bufs=3, space="PSUM") as pspool,
        ):
            # weights first, on the fast sync queue -- the scalar queue
            # starts late (ACT table load) and stalled the first m-tiles
            ws = []
            for k in range(KT):
                wt = wpool.tile([128, 512], _F16, tag=f"w{k}", name=f"w{k}")
                nc.sync.dma_start(wt[:], w_d[k][:])
                ws.append(wt)
            # short HAM warm-up bridge: PE busy while the first loads land
            jx = wpool.tile([128, 512], _F16, tag="jx", name="jx")
            nc.gpsimd.memset(jx[:], 0.25)
            spin = pspool.tile([128, 512], _F32, tag="spin", name="spin", bufs=1)
            for _ in range(NSPIN):
                nc.tensor.matmul(spin[:], jx[:, 0:128], jx[:], start=True, stop=True)
            for mt2 in range(MT // MP):
                xs = xpool.tile([128, MP * 8 * 128], _F16, tag="xs")
                nc.sync.dma_start(xs[:], xt_d[mt2][:])
                for half in range(MP):
                    mt = mt2 * MP + half
                    if mt % MS == 0:
                        ot = opool.tile([128, MS * OW], _F16, tag="ot")
                    oo = (mt % MS) * OW
                    xb = half * 8 * 128
                    pa = pspool.tile([128, 512], _F32, tag="pa", name="pa")
                    pb = pspool.tile([128, 512], _F32, tag="pb", name="pb")
                    for k in range(KT):
                        nc.tensor.matmul(
                            pa[:],
                            xs[:, xb + k * 128 : xb + (k + 1) * 128],
                            ws[k][:],
                            start=(k == 0),
                            stop=(k == KT - 1),
                        )
                    for k in range(KT):
                        nc.tensor.matmul(
                            pb[:],
                            xs[:, xb + (4 + k) * 128 : xb + (5 + k) * 128],
                            ws[k][:],
                            start=(k == 0),
                            stop=(k == KT - 1),
                        )
                    # evacuate PSUM -> fp16 store tile; ACT takes PA, DVE PB
                    nc.scalar.activation(ot[:, oo : oo + 512], pa[:], _copy, bias=0.0)
                    nc.vector.tensor_scalar_add(ot[:, oo + 512 : oo + 1024], pb[:], 0.0)
                    if mt % MS == MS - 1:
                        nc.gpsimd.dma_start(out_d[mt // MS][:], ot[:])
    nc.compile()
    return nc


_cached = {}


def _get_bass(trace=False):
    if "nc" not in _cached:
        _cached["nc"] = _build_bass()
    return _cached["nc"]


def _prep_weights(w_re, w_im):
    w_re = np.asarray(w_re, np.float32)
    w_im = np.asarray(w_im, np.float32)
    rhs = np.empty((K, 512), np.float16)
    rhs[:, 0:257] = w_re[0:257].T
    rhs[:, 257:512] = w_im[1:256].T
    return np.ascontiguousarray(rhs.reshape(KT, 128, 512))


def _prep_x_core(xr, xi):
    # lhsT: xcat_t[k, m] with k = [re 512 | im 512]; tile to
    # (MT/2 pairs, 128 k-in-block, (half, kblock, 128 m))
    xcat_t = np.empty((2 * N, M), np.float16)
    xcat_t[:N] = xr.reshape(M, N).T
    xcat_t[N:] = xi.reshape(M, N).T
    xt = xcat_t.reshape(8, 128, MT // MP, MP, 128).transpose(2, 1, 3, 0, 4)
    return np.ascontiguousarray(xt).reshape(MT // MP, 128, MP * 8 * 128)


def kernel(x_re, x_im, w_re, w_im, _trace=False, _trace_kwargs=None):
    x_re = np.asarray(x_re, np.float32)
    x_im = np.asarray(x_im, np.float32)
    w = _prep_weights(w_re, w_im)
    in_maps = [
        {"xt": _prep_x_core(x_re[c], x_im[c]), "w": w} for c in range(B)
    ]
    nc = _get_bass(_trace)
    res = run_bass_kernel_spmd(
        nc, in_maps, list(range(B)), trace=_trace, **(_trace_kwargs or {})
    )
    # edge spectrum cols (h=0, 256) in full f32 on host: plain/alternating sums
    alt = np.empty(N, np.float32)
    alt[0::2] = 1.0
    alt[1::2] = -1.0
    e0_re = x_re.sum(-1)                  # (B,16,256)
    e0_im = x_im.sum(-1)
    e256_re = x_re @ alt
    e256_im = x_im @ alt
    out = np.empty((B, 16, 256, N, 2), np.float32)
    for c in range(B):
        slab = (
            res.results[c]["out"]
            .reshape(MT // MS, 128, MS, OW)
            .transpose(0, 2, 1, 3)
            .reshape(M, OW)
            .astype(np.float32)
        )
        # PA = [P1(0..256) | Q1(1..255)], PB = [Q2(0..256) | P2(1..255)]
        P1 = slab[:, 0:257]
        Q1 = slab[:, 257:512]
        Q2 = slab[:, 512:769]
        P2 = slab[:, 769:1024]
        y = np.empty((M, N, 2), np.float32)
        y[:, 1:256, 0] = P1[:, 1:256] - P2
        y[:, 1:256, 1] = Q1 + Q2[:, 1:256]
        y[:, 257:512, 0] = (P1[:, 1:256] + P2)[:, ::-1]
        y[:, 257:512, 1] = (Q2[:, 1:256] - Q1)[:, ::-1]
        y[:, 0, 0] = e0_re[c].ravel()
        y[:, 0, 1] = e0_im[c].ravel()
        y[:, 256, 0] = e256_re[c].ravel()
        y[:, 256, 1] = e256_im[c].ravel()
        out[c] = y.reshape(16, 256, N, 2)
    if _trace:
        kernel._last_result = res
    return out


# revision 9
# speedup vs baseline: 1.0621x; 1.0355x over previous
"""Complex DFT (512-pt) over rows of x = x_re + i*x_im, y = x @ W^T (complex).

Full inputs: x_re, x_im (8,16,256,512) f32; w_re, w_im (512,512) f32.
Full output: (8,16,256,512,2) f32  (re/im interleaved on last axis).

Strategy (fp16 on-device; tolerance gate is 2e-2, fp16 lands ~5e-4):
  Conjugate symmetry W[N-h] = conj(W[h]) -> only half the spectrum columns.
  The four half-spectrum real products collapse into TWO 512-wide matmuls
  against ONE shared rhs:
      RHS = [ C(h=0..256) | D(h=1..255) ]   (512 cols = exactly 1 PSUM bank)
      PA  = Xre @ RHS = [ P1(0..256) | Q1(1..255) ]
      PB  = Xim @ RHS = [ Q2(0..256) | P2(1..255) ]
  with C = Re(W), D = Im(W), P1=A@C, P2=B@D, Q1=A@D, Q2=B@C.
  The device stores PA|PB raw (fp16); the HOST does the +- spectrum combines
  in f32 (same bytes, same accuracy as an on-device combine after the fp16
  round).  Edge columns h=0 and h=256 are row-sums of x computed on host.

  Per core (batch-sharded): 256 matmuls of [128k,128m]^T @ [128k,512] -> 55.3us
  PE roofline at 216 ns/MM (HW-measured).  Scheduling details that matter:
   - x is ONE flat [128, 32*1024] dram tensor; loads are free-dim slices with
     sizes [1,1,2,4,...]*1024 cols: the first matmul only waits on a 256 KB
     load, while later 1 MB loads amortize the ~600 ns per-DMA issue cost on
     the Sync sequencer.
   - weights load first on the same queue (the scalar queue starts ~5us late).
   - 8 junk matmuls bridge the first-load latency so the PE_HAM clock gate
     (1.2 GHz cold -> 2.4 GHz after 3.4us of gapless activity) latches warm
     early and never relapses.
   - ACT evacuates PA, DVE evacuates PB (fp16 into the store tile), gpsimd
     stores 2-m-tile mega-tiles.  PSUM: pa/pb tags x4 bufs = all 8 banks.
"""

import sys

sys.path.insert(0, "/opt/trn_rl_repo")

import numpy as np

import concourse.bass as bass
import concourse.mybir as mybir
import concourse.tile as tile
from concourse import bacc
from concourse.bass_utils import run_bass_kernel_spmd

N = 512          # DFT size
B = 8            # batch -> one per core
M = 4096         # rows per core (16*256)
K = N            # contraction per product
KT = K // 128    # 4 k-subtiles per product
MT = M // 128    # 32 m-tiles
MS = 2           # m-tiles per output DMA (mega-store)
OW = 1024        # output cols per m-tile (PA | PB)
NSPIN = 8        # warm-up matmuls (bridge first-load latency)
LOAD_SPLIT = [1, 1, 2, 4, 4, 4, 4, 4, 4, 4]   # m-tiles per input DMA
assert sum(LOAD_SPLIT) == MT

_F32 = mybir.dt.float32
_F16 = mybir.dt.float16


def _build_bass():
    nc = bacc.Bacc("TRN2", target_bir_lowering=False, debug=False, num_devices=B)
    # x: flat [128 k-in-block, (mt, kblock, 128 m)]; kblocks 0..3 = x_re,
    # 4..7 = x_im (each [128 k-in-block, 128 m] lhsT stripe).
    xt_d = nc.dram_tensor("xt", [128, MT * 8 * 128], _F16, kind="ExternalInput")
    w_d = nc.dram_tensor("w", [KT, 128, 512], _F16, kind="ExternalInput")
    out_d = nc.dram_tensor("out", [MT // MS, 128, MS * OW], _F16, kind="ExternalOutput")

    _copy = mybir.ActivationFunctionType.Copy

    with tile.TileContext(nc) as tc:
        with (
            tc.tile_pool(name="wpool", bufs=1) as wpool,
            tc.tile_pool(name="xpool", bufs=1) as xpool,
            tc.tile_pool(name="opool", bufs=4) as opool,
            tc.tile_pool(name="psum", bufs=4, space="PSUM") as pspool,
        ):
            # weights first on the sync queue
            ws = []
            for k in range(KT):
                wt = wpool.tile([128, 512], _F16, tag=f"w{k}", name=f"w{k}")
                nc.sync.dma_start(wt[:], w_d[k][:])
                ws.append(wt)
            # HAM warm-up bridge (PE busy while the first loads land)
            jx = wpool.tile([128, 512], _F16, tag="jx", name="jx")
            nc.gpsimd.memset(jx[:], 0.25)
            spin = pspool.tile([128, 512], _F32, tag="pa", name="spin")
            for _ in range(NSPIN):
                nc.tensor.matmul(spin[:], jx[:, 0:128], jx[:], start=True, stop=True)
            # prefetch every x slice (all tiles stay resident: 64 KB/partition)
            xtiles = []   # (xs_tile, local m-tile index) per global m-tile
            off = 0
            for g, nmt in enumerate(LOAD_SPLIT):
                xs = xpool.tile([128, nmt * 8 * 128], _F16, tag=f"xs{g}", name=f"xs{g}")
                nc.sync.dma_start(xs[:], xt_d[:, off * 8 * 128 : (off + nmt) * 8 * 128])
                for l in range(nmt):
                    xtiles.append((xs, l))
                off += nmt
            for mt in range(MT):
                xs, l = xtiles[mt]
                if mt % MS == 0:
                    ot = opool.tile([128, MS * OW], _F16, tag="ot")
                oo = (mt % MS) * OW
                xb = l * 8 * 128
                pa = pspool.tile([128, 512], _F32, tag="pa", name="pa")
                pb = pspool.tile([128, 512], _F32, tag="pb", name="pb")
                for k in range(KT):
                    nc.tensor.matmul(
                        pa[:],
                        xs[:, xb + k * 128 : xb + (k + 1) * 128],
                        ws[k][:],
                        start=(k == 0),
                        stop=(k == KT - 1),
                    )
                for k in range(KT):
                    nc.tensor.matmul(
                        pb[:],
                        xs[:, xb + (4 + k) * 128 : xb + (5 + k) * 128],
                        ws[k][:],
                        start=(k == 0),
                        stop=(k == KT - 1),
                    )
                # evacuate PSUM -> fp16 store tile; ACT takes PA, DVE PB
                nc.scalar.activation(ot[:, oo : oo + 512], pa[:], _copy, bias=0.0)
                nc.vector.tensor_scalar_add(ot[:, oo + 512 : oo + 1024], pb[:], 0.0)
                if mt % MS == MS - 1:
                    nc.gpsimd.dma_start(out_d[mt // MS][:], ot[:])
    nc.compile()
    return nc


_cached = {}


def _get_bass(trace=False):
    if "nc" not in _cached:
        _cached["nc"] = _build_bass()
    return _cached["nc"]


def _prep_weights(w_re, w_im):
    w_re = np.asarray(w_re, np.float32)
    w_im = np.asarray(w_im, np.float32)
    rhs = np.empty((K, 512), np.float16)
    rhs[:, 0:257] = w_re[0:257].T
    rhs[:, 257:512] = w_im[1:256].T
    return np.ascontiguousarray(rhs.reshape(KT, 128, 512))


def _prep_x_core(xr, xi):
    # lhsT: xcat_t[k, m] with k = [re 512 | im 512]; flat layout
    # [128 k-in-block, (mt, kblock, 128 m)]
    xcat_t = np.empty((2 * N, M), np.float16)
    xcat_t[:N] = xr.reshape(M, N).T
    xcat_t[N:] = xi.reshape(M, N).T
    xt = xcat_t.reshape(8, 128, MT, 128).transpose(1, 2, 0, 3)
    return np.ascontiguousarray(xt).reshape(128, MT * 8 * 128)


def kernel(x_re, x_im, w_re, w_im, _trace=False, _trace_kwargs=None):
    x_re = np.asarray(x_re, np.float32)
    x_im = np.asarray(x_im, np.float32)
    w = _prep_weights(w_re, w_im)
    in_maps = [
        {"xt": _prep_x_core(x_re[c], x_im[c]), "w": w} for c in range(B)
    ]
    nc = _get_bass(_trace)
    res = run_bass_kernel_spmd(
        nc, in_maps, list(range(B)), trace=_trace, **(_trace_kwargs or {})
    )
    # edge spectrum cols (h=0, 256) in full f32 on host: plain/alternating sums
    alt = np.empty(N, np.float32)
    alt[0::2] = 1.0
    alt[1::2] = -1.0
    e0_re = x_re.sum(-1)                  # (B,16,256)
    e0_im = x_im.sum(-1)
    e256_re = x_re @ alt
    e256_im = x_im @ alt
    out = np.empty((B, 16, 256, N, 2), np.float32)
    for c in range(B):
        slab = (
            res.results[c]["out"]
            .reshape(MT // MS, 128, MS, OW)
            .transpose(0, 2, 1, 3)
            .reshape(M, OW)
            .astype(np.float32)
        )
        # PA = [P1(0..256) | Q1(1..255)], PB = [Q2(0..256) | P2(1..255)]
        P1 = slab[:, 0:257]
        Q1 = slab[:, 257:512]
        Q2 = slab[:, 512:769]
        P2 = slab[:, 769:1024]
        y = np.empty((M, N, 2), np.float32)
        y[:, 1:256, 0] = P1[:, 1:256] - P2
        y[:, 1:256, 1] = Q1 + Q2[:, 1:256]
        y[:, 257:512, 0] = (P1[:, 1:256] + P2)[:, ::-1]
        y[:, 257:512, 1] = (Q2[:, 1:256] - Q1)[:, ::-1]
        y[:, 0, 0] = e0_re[c].ravel()
        y[:, 0, 1] = e0_im[c].ravel()
        y[:, 256, 0] = e256_re[c].ravel()
        y[:, 256, 1] = e256_im[c].ravel()
        out[c] = y.reshape(16, 256, N, 2)
    if _trace:
        kernel._last_result = res
    return out
